# revision 5
# baseline (speedup 1.0000x reference)
"""Trainium2 Bass kernel for AnsiToPixels (embedding_lookup, memory-bound).

Computation (per glyph cell):
  raw[y,x]  = sum_ch char[ch] * glyph[ch,y,x]          (256-ch dense "one-hot" matmul)
  fg[c]     = (0.5*fg_bold+0.5) * fg_color[c]
  bg[c]     = (0.5*bg_bold+0.5) * bg_color[c]
  out[y,x,c] = raw[y,x]*(fg[c]-bg[c]) + bg[c]

Sharding: pure data parallelism over batch B=128 -> 16 per core on 8 cores,
glyph table replicated. Each core processes 25600 cells.

Design (v3; v2 measured ~139-143us, f32 v1 ~217us):
  - Host uploads the 256 glyph channels CHANNEL-MAJOR as fp8e4m3
    ([chmod, half, t, j, p]) so the kernel needs NO PE transposes and no
    PSUM->SBUF ct copies; the 8 color channels ride separately as fp16
    cell-major. Input HBM traffic drops to ~7 MB/core (from 13.5 fp16 /
    27 f32); output stays fp16 (~19.7 MB/core).
  - The glyph lookup is ONE DoubleRow fp8 matmul per 128-cell column
    (K=256 folded as 2 k-tiles, 2x PE throughput): 200 matmuls/core,
    ~27ns each - PE is negligible.
  - The per-(cell,channel) affine blend out = raw*d + bgs runs as
    per-partition-scalar ops reading raw DIRECTLY from f32 PSUM (no
    intermediate copy): tensor_scalar on DVE/Pool, activation on ACT,
    600 instrs/core split to equalize engine busy time.
  - Color math (d = fg*sf - bg*sb, bgs) runs on Pool one tile ahead so
    blends never wait on it.
  - Output DRAM layout mirrors SBUF ([tile, p, j, c, y, x] fp16), one
    contiguous-per-partition DMA per 4-cell group; host does the final
    permutation to [B, 320, 640, 3] and upcasts to f32.
"""

import os
import sys

import numpy as np

for _p in ("/opt/trn_rl_repo", "/root/.axon_site/_ro/trn_rl_repo"):
    if os.path.isdir(_p) and _p not in sys.path:
        sys.path.insert(0, _p)

import ml_dtypes  # noqa: E402

import concourse.bass as bass  # noqa: E402
import concourse.mybir as mybir  # noqa: E402
import concourse.tile as tile  # noqa: E402
from concourse import bacc  # noqa: E402
from concourse.bass_utils import run_bass_kernel_spmd  # noqa: E402


def _ensure_ntff_hook():
    """Register the axon NTFF profile hook if the image's antenv lacks it,
    so run_bass_kernel_spmd(trace=True) can capture HW exec time."""
    try:
        from antenv.axon_hooks import get_axon_ntff_profile_hook  # noqa: F401

        return
    except ImportError:
        pass
    try:
        import types

        import antenv
        from trn_agent_boot.trn_boot import _ntff_profile_via_ctypes

        hook = _ntff_profile_via_ctypes("/opt/axon/libaxon_pjrt.so")
        mod = types.ModuleType("antenv.axon_hooks")
        mod.get_axon_ntff_profile_hook = lambda: hook
        mod.set_axon_ntff_profile_hook = lambda h: None
        sys.modules["antenv.axon_hooks"] = mod
        antenv.axon_hooks = mod
    except Exception as e:  # profiling is best-effort
        print(f"NTFF hook registration failed: {e}", file=sys.stderr)


N_CORES = 8
B = 128
GRID_H, GRID_W = 20, 80
GLYPH_H, GLYPH_W = 16, 8
N_GLYPHS = 256
PIX = GLYPH_H * GLYPH_W  # 128

B_SHARD = B // N_CORES  # 16
CELLS = B_SHARD * GRID_H * GRID_W  # 25600
OCT = 20  # cells per partition per macro-tile
MT = 128 * OCT  # cells per macro-tile (2560)
NT = CELLS // MT  # 10 macro-tiles
NG = OCT // 4  # j-groups per macro-tile (5)

F32 = mybir.dt.float32
F16 = mybir.dt.float16
F8 = mybir.dt.float8e4

# blend engine assignment per group: 12 slots (jj, c) -> engine.
# DVE reads the fp16 SBUF copy at 4x (163ns model), Pool reads fp16 SBUF
# (GPSIMD can't touch PSUM), ACT reads f32 PSUM directly (no copy dep).
BLEND_ENGINES = {
    (0, 0): "v", (0, 1): "v", (0, 2): "v",
    (1, 0): "v", (1, 1): "v", (1, 2): "v",
    (2, 0): "p", (2, 1): "p", (2, 2): "p",
    (3, 0): "p", (3, 1): "a", (3, 2): "a",
}


def _bcast_last(ap, n):
    """Append a stride-0 dim of size n to an AP (free-dim broadcast)."""
    return bass.AP(tensor=ap.tensor, offset=ap.offset, ap=[*ap.ap, [0, n]])


def build_kernel():
    nc = bacc.Bacc(
        "TRN2",
        target_bir_lowering=False,
        debug=False,
        enable_asserts=False,
        num_devices=N_CORES,
    )
    # glyph channels, channel-major fp8: [chmod, half, t, j, pcol]
    data8 = nc.dram_tensor(
        "data8", [128, 2, NT, OCT, 128], F8, kind="ExternalInput"
    ).ap()
    # color channels, cell-major fp16: [t, p, j, 8]
    datac = nc.dram_tensor("datac", [NT, 128, OCT, 8], F16, kind="ExternalInput").ap()
    # glyph table fp8: [chmod, half, pix]
    glyph8 = nc.dram_tensor("glyph8", [128, 2, PIX], F8, kind="ExternalInput").ap()
    outp = nc.dram_tensor(
        "out", [NT, 128, OCT, 3, PIX], F16, kind="ExternalOutput"
    ).ap()

    with tile.TileContext(nc) as tc:
        with (
            tc.tile_pool(name="const", bufs=1) as const,
            tc.tile_pool(name="ct", bufs=3) as ct_pool,
            tc.tile_pool(name="cc", bufs=3) as cc_pool,
            tc.tile_pool(name="grp", bufs=3) as grp_pool,
            tc.tile_pool(name="raw", bufs=3) as raw_pool,
            tc.tile_pool(name="outsb", bufs=3) as out_pool,
            tc.tile_pool(name="psR", bufs=6, space="PSUM") as psR,
        ):
            g8 = const.tile([128, 2, PIX], F8)
            nc.sync.dma_start(out=g8[:, :, :], in_=glyph8[:, :, :])

            cts = {}
            ccs = {}

            def load_tile(t):
                cts[t] = ct_pool.tile([128, 2, OCT, 128], F8, name="ct", tag="ct")
                nc.gpsimd.dma_start(out=cts[t][:, :, :, :], in_=data8[:, :, t, :, :])
                ccs[t] = cc_pool.tile([128, OCT, 8], F16, name="cc", tag="cc")
                nc.gpsimd.dma_start(out=ccs[t][:, :, :], in_=datac[t, :, :, :])

            colors = {}

            def color_math(t):
                """d = fg*sf - bg*sb; bgs = bg*sb (Pool, one tile ahead)."""
                cc = ccs[t]
                sf = grp_pool.tile([128, OCT], F32, tag="sf")
                sb = grp_pool.tile([128, OCT], F32, tag="sb")
                fg = grp_pool.tile([128, OCT, 3], F32, tag="fg")
                bgs = grp_pool.tile([128, OCT, 3], F32, tag="bgs")
                d = grp_pool.tile([128, OCT, 3], F32, tag="d")
                nc.gpsimd.tensor_scalar(
                    out=sf[:, :],
                    in0=cc[:, :, 0],
                    scalar1=0.5,
                    scalar2=0.5,
                    op0=mybir.AluOpType.mult,
                    op1=mybir.AluOpType.add,
                )
                nc.gpsimd.tensor_scalar(
                    out=sb[:, :],
                    in0=cc[:, :, 4],
                    scalar1=0.5,
                    scalar2=0.5,
                    op0=mybir.AluOpType.mult,
                    op1=mybir.AluOpType.add,
                )
                nc.gpsimd.tensor_mul(
                    fg[:, :, :], cc[:, :, 1:4], _bcast_last(sf[:, :], 3)
                )
                nc.gpsimd.tensor_mul(
                    bgs[:, :, :], cc[:, :, 5:8], _bcast_last(sb[:, :], 3)
                )
                nc.gpsimd.tensor_sub(d[:, :, :], fg[:, :, :], bgs[:, :, :])
                colors[t] = (d, bgs)

            load_tile(0)
            color_math(0)
            load_tile(1)

            for t in range(NT):
                if t + 1 < NT:
                    color_math(t + 1)
                if t + 2 < NT:
                    load_tile(t + 2)
                ct = cts.pop(t)
                ccs.pop(t)
                d, bgs = colors.pop(t)

                out_sb = out_pool.tile([128, OCT, 3, PIX], F16)

                for g in range(NG):
                    j0 = 4 * g
                    rawp = psR.tile([128, 4 * PIX], F32)
                    for jj in range(4):
                        nc.tensor.matmul(
                            rawp[:, jj * PIX : (jj + 1) * PIX],
                            ct[:, :, j0 + jj, :],
                            g8[:, :, :],
                            start=True,
                            stop=True,
                            perf_mode=mybir.MatmulPerfMode.DoubleRow,
                        )
                    # group copy+cast PSUM raw f32 -> SBUF fp16 (ACT)
                    raw16 = raw_pool.tile([128, 4 * PIX], F16)
                    nc.scalar.copy(raw16[:, :], rawp[:, :])
                    # blends: out = raw*d + bgs
                    for jj in range(4):
                        j = j0 + jj
                        for c in range(3):
                            eng = BLEND_ENGINES[(jj, c)]
                            if eng == "a":
                                nc.scalar.activation(
                                    out_sb[:, j, c, :],
                                    rawp[:, jj * PIX : (jj + 1) * PIX],
                                    mybir.ActivationFunctionType.Identity,
                                    bias=bgs[:, j, c : c + 1],
                                    scale=d[:, j, c : c + 1],
                                )
                            else:
                                e = nc.vector if eng == "v" else nc.gpsimd
                                e.tensor_scalar(
                                    out=out_sb[:, j, c, :],
                                    in0=raw16[:, jj * PIX : (jj + 1) * PIX],
                                    scalar1=d[:, j, c : c + 1],
                                    scalar2=bgs[:, j, c : c + 1],
                                    op0=mybir.AluOpType.mult,
                                    op1=mybir.AluOpType.add,
                                )
                    nc.sync.dma_start(
                        out=outp[t, :, j0 : j0 + 4, :, :],
                        in_=out_sb[:, j0 : j0 + 4, :, :],
                    )

    nc.compile()
    return nc


_NC = None


def _get_nc():
    global _NC
    if _NC is None:
        _NC = build_kernel()
    return _NC


def run(data, char_matrix, trace=False):
    data = np.ascontiguousarray(np.asarray(data, dtype=np.float32))
    glyph = np.asarray(char_matrix, dtype=np.float32).reshape(N_GLYPHS, PIX)
    assert data.shape == (B, GRID_H, GRID_W, 264), data.shape

    f8 = ml_dtypes.float8_e4m3
    # glyph table [256, 128] -> [chmod, half, pix] fp8
    g8 = np.ascontiguousarray(
        glyph.reshape(2, 128, PIX).transpose(1, 0, 2).astype(f8)
    )

    in_maps = []
    for i in range(N_CORES):
        shard = data[i * B_SHARD : (i + 1) * B_SHARD].reshape(CELLS, 264)
        # glyph channels: [t, p, j, half, chmod] -> [chmod, half, t, j, p] fp8
        gl = shard[:, :256].astype(f8).reshape(NT, 128, OCT, 2, 128)
        d8 = np.ascontiguousarray(gl.transpose(4, 3, 0, 2, 1))
        # color channels: [t, p, j, 8] fp16
        cc = np.ascontiguousarray(
            shard[:, 256:264].astype(np.float16).reshape(NT, 128, OCT, 8)
        )
        in_maps.append({"data8": d8, "datac": cc, "glyph8": g8})

    nc = _get_nc()
    if trace:
        _ensure_ntff_hook()
    res = run_bass_kernel_spmd(
        nc, in_maps, core_ids=list(range(N_CORES)), trace=trace
    )
    outs = []
    for r in res.results:
        # [t, p, j, c, y, x] -> [t, rg, o, j, c, y, x] -> [t, rg, y, o, j, x, c]
        o = r["out"].reshape(NT, 32, 4, OCT, 3, GLYPH_H, GLYPH_W)
        o = o.transpose(0, 1, 5, 2, 3, 6, 4).astype(np.float32)
        # rows r = t*32+rg = (b, h); W = o*160 + j*8 + x
        o = o.reshape(B_SHARD, GRID_H * GLYPH_H, GRID_W * GLYPH_W, 3)
        outs.append(o)
    out = np.concatenate(outs, axis=0)
    return out, res.exec_time_ns


def kernel(data, char_matrix):
    out, _ = run(data, char_matrix, trace=False)
    return out


# revision 7
# speedup vs baseline: 1.1890x; 1.1890x over previous
"""Trainium2 Bass kernel for AnsiToPixels (embedding_lookup, memory-bound).

Computation (per glyph cell):
  raw[y,x]  = sum_ch char[ch] * glyph[ch,y,x]          (256-ch dense "one-hot" matmul)
  fg[c]     = (0.5*fg_bold+0.5) * fg_color[c]
  bg[c]     = (0.5*bg_bold+0.5) * bg_color[c]
  out[y,x,c] = raw[y,x]*(fg[c]-bg[c]) + bg[c]

Sharding: pure data parallelism over batch B=128 -> 16 per core on 8 cores,
glyph table replicated. Each core processes 25600 cells.

Design notes (v4):
  - Host uploads the 256 glyph channels CHANNEL-MAJOR as fp8e4m3
    ([chmod, half, t, j, p]) so the kernel needs NO PE transposes and no
    PSUM->SBUF ct copies; the 8 color channels ride separately as fp16
    cell-major. Input HBM traffic ~7 MB/core; output fp16 ~19.7 MB/core.
  - Glyph lookup: ONE DoubleRow fp8 matmul per 128-cell column (K=256 as
    2 k-tiles), all 20 matmuls of a macro-tile emitted back-to-back so
    the PE pipelines and ramps its clock.
  - Blend out = raw*d + bgs is assigned per 4-cell GROUP to one engine
    to minimize cross-engine semaphore traffic (the NC's activity
    throttle clamps utilization to 50% for most of the kernel, making
    per-instruction overhead the dominant cost):
      * group 0 of each tile: ACT, 12 small per-(j,c) activations
        reading f32 PSUM directly (scale=d, bias=bgs).
      * groups 1-4: DVE big tensor_tensor mult (raw bcast over c x d
        bcast over pix) from PSUM into fp16 out_sb, then an in-place
        broadcast add of bgs on Pool (groups 1,2[,3]) or DVE (rest).
    This is ~2 big instrs/group instead of 12 small ones.
  - Color math (d = fg*sf - bg*sb, bgs) is 3 merged Pool ops per tile,
    computed one tile ahead; d/bgs stored fp16.
  - Output DRAM layout mirrors SBUF ([tile, p, j, c, y, x] fp16), one
    DMA per group; host does the final permutation to [B, 320, 640, 3].
"""

import os
import sys

import numpy as np

for _p in ("/opt/trn_rl_repo", "/root/.axon_site/_ro/trn_rl_repo"):
    if os.path.isdir(_p) and _p not in sys.path:
        sys.path.insert(0, _p)

import ml_dtypes  # noqa: E402

import concourse.bass as bass  # noqa: E402
import concourse.mybir as mybir  # noqa: E402
import concourse.tile as tile  # noqa: E402
from concourse import bacc  # noqa: E402
from concourse.bass_utils import run_bass_kernel_spmd  # noqa: E402


def _ensure_ntff_hook():
    """Register the axon NTFF profile hook if the image's antenv lacks it,
    so run_bass_kernel_spmd(trace=True) can capture HW exec time."""
    try:
        from antenv.axon_hooks import get_axon_ntff_profile_hook  # noqa: F401

        return
    except ImportError:
        pass
    try:
        import types

        import antenv
        from trn_agent_boot.trn_boot import _ntff_profile_via_ctypes

        hook = _ntff_profile_via_ctypes("/opt/axon/libaxon_pjrt.so")
        mod = types.ModuleType("antenv.axon_hooks")
        mod.get_axon_ntff_profile_hook = lambda: hook
        mod.set_axon_ntff_profile_hook = lambda h: None
        sys.modules["antenv.axon_hooks"] = mod
        antenv.axon_hooks = mod
    except Exception as e:  # profiling is best-effort
        print(f"NTFF hook registration failed: {e}", file=sys.stderr)


N_CORES = 8
B = 128
GRID_H, GRID_W = 20, 80
GLYPH_H, GLYPH_W = 16, 8
N_GLYPHS = 256
PIX = GLYPH_H * GLYPH_W  # 128

B_SHARD = B // N_CORES  # 16
CELLS = B_SHARD * GRID_H * GRID_W  # 25600
OCT = 20  # cells per partition per macro-tile
MT = 128 * OCT  # cells per macro-tile (2560)
NT = CELLS // MT  # 10 macro-tiles
NG = OCT // 4  # j-groups per macro-tile (5)

F32 = mybir.dt.float32
F16 = mybir.dt.float16
F8 = mybir.dt.float8e4


def _view(ap, dims):
    """AP with the partition dim of `ap` and explicit free dims [(stride, n)].
    Strides are in elements of the tile dtype; offset comes from `ap`."""
    return bass.AP(
        tensor=ap.tensor, offset=ap.offset, ap=[ap.ap[0], *[[s, n] for s, n in dims]]
    )


def build_kernel():
    nc = bacc.Bacc(
        "TRN2",
        target_bir_lowering=False,
        debug=False,
        enable_asserts=False,
        num_devices=N_CORES,
    )
    # glyph channels, channel-major fp8: [chmod, half, t, j, pcol]
    data8 = nc.dram_tensor(
        "data8", [128, 2, NT, OCT, 128], F8, kind="ExternalInput"
    ).ap()
    # color channels, cell-major fp16: [t, p, j, 8]
    datac = nc.dram_tensor("datac", [NT, 128, OCT, 8], F16, kind="ExternalInput").ap()
    # glyph table fp8: [chmod, half, pix]
    glyph8 = nc.dram_tensor("glyph8", [128, 2, PIX], F8, kind="ExternalInput").ap()
    outp = nc.dram_tensor(
        "out", [NT, 128, OCT, 3, PIX], F16, kind="ExternalOutput"
    ).ap()

    with tile.TileContext(nc) as tc:
        with (
            tc.tile_pool(name="const", bufs=1) as const,
            tc.tile_pool(name="ct", bufs=3) as ct_pool,
            tc.tile_pool(name="cc", bufs=3) as cc_pool,
            tc.tile_pool(name="grp", bufs=3) as grp_pool,
            tc.tile_pool(name="outsb", bufs=3) as out_pool,
            tc.tile_pool(name="psR", bufs=6, space="PSUM") as psR,
        ):
            g8 = const.tile([128, 2, PIX], F8)
            nc.sync.dma_start(out=g8[:, :, :], in_=glyph8[:, :, :])

            cts = {}
            ccs = {}

            def load_tile(t):
                cts[t] = ct_pool.tile([128, 2, OCT, 128], F8, name="ct", tag="ct")
                nc.sync.dma_start(out=cts[t][:, :, :, :], in_=data8[:, :, t, :, :])
                ccs[t] = cc_pool.tile([128, OCT, 8], F16, name="cc", tag="cc")
                nc.sync.dma_start(out=ccs[t][:, :, :], in_=datac[t, :, :, :])

            colors = {}

            def color_math(t):
                """d = fg*sf - bg*sb; bgs = bg*sb. 3 merged Pool ops."""
                cc = ccs[t]
                sfsb = grp_pool.tile([128, OCT, 2], F32, tag="sfsb")
                fgbg = grp_pool.tile([128, OCT, 2, 3], F32, tag="fgbg")
                d = grp_pool.tile([128, OCT, 3], F32, tag="d")
                # sfsb[:, j, h] = 0.5*cc[:, j, 4h] + 0.5
                nc.gpsimd.tensor_scalar(
                    out=sfsb[:, :, :],
                    in0=_view(cc[:, :, :], [(8, OCT), (4, 2)]),
                    scalar1=0.5,
                    scalar2=0.5,
                    op0=mybir.AluOpType.mult,
                    op1=mybir.AluOpType.add,
                )
                # fgbg[:, j, h, c] = cc[:, j, 1+4h+c] * sfsb[:, j, h]
                cc1 = cc[:, :, 1:4]  # offset anchor
                nc.gpsimd.tensor_mul(
                    fgbg[:, :, :, :],
                    _view(cc1, [(8, OCT), (4, 2), (1, 3)]),
                    _view(sfsb[:, :, :], [(2, OCT), (1, 2), (0, 3)]),
                )
                # d = fg_s - bg_s
                nc.gpsimd.tensor_sub(d[:, :, :], fgbg[:, :, 0, :], fgbg[:, :, 1, :])
                colors[t] = (d, fgbg)

            load_tile(0)
            color_math(0)
            load_tile(1)

            for t in range(NT):
                if t + 1 < NT:
                    color_math(t + 1)
                if t + 2 < NT:
                    load_tile(t + 2)
                ct = cts.pop(t)
                ccs.pop(t)
                d, fgbg = colors.pop(t)
                # bgs lives inside fgbg at [:, j, 1, c]

                out_sb = out_pool.tile([128, OCT, 3, PIX], F16)

                # all 20 matmuls back-to-back so the PE pipelines/ramps
                rawps = []
                for g in range(NG):
                    rawp = psR.tile([128, 4 * PIX], F32)
                    rawps.append(rawp)
                    j0 = 4 * g
                    for jj in range(4):
                        nc.tensor.matmul(
                            rawp[:, jj * PIX : (jj + 1) * PIX],
                            ct[:, :, j0 + jj, :],
                            g8[:, :, :],
                            start=True,
                            stop=True,
                            perf_mode=mybir.MatmulPerfMode.DoubleRow,
                        )

                for g in range(NG):
                    j0 = 4 * g
                    rawp = rawps[g]
                    out_g = out_sb[:, j0 : j0 + 4, :, :]
                    if g == 0:
                        # ACT group: 12 small activations from PSUM
                        for jj in range(4):
                            j = j0 + jj
                            for c in range(3):
                                nc.scalar.activation(
                                    out_sb[:, j, c, :],
                                    rawp[:, jj * PIX : (jj + 1) * PIX],
                                    mybir.ActivationFunctionType.Identity,
                                    bias=fgbg[:, j, 1, c : c + 1],
                                    scale=d[:, j, c : c + 1],
                                )
                    else:
                        # pass1 (DVE): out = raw (bcast c) * d (bcast pix)
                        nc.vector.tensor_mul(
                            out_g,
                            _view(rawp[:, :], [(PIX, 4), (0, 3), (1, PIX)]),
                            _view(d[:, j0 : j0 + 4, :], [(3, 4), (1, 3), (0, PIX)]),
                        )
                        # pass2: out += bgs (bcast pix), Pool or DVE
                        e = nc.gpsimd if (g <= 2 if t % 2 == 0 else g <= 1) else nc.vector
                        e.tensor_add(
                            out_g,
                            out_g,
                            _view(
                                fgbg[:, j0 : j0 + 4, 1, :], [(6, 4), (1, 3), (0, PIX)]
                            ),
                        )
                    nc.sync.dma_start(
                        out=outp[t, :, j0 : j0 + 4, :, :],
                        in_=out_g,
                    )

    nc.compile()
    return nc


_NC = None


def _get_nc():
    global _NC
    if _NC is None:
        _NC = build_kernel()
    return _NC


def run(data, char_matrix, trace=False):
    data = np.ascontiguousarray(np.asarray(data, dtype=np.float32))
    glyph = np.asarray(char_matrix, dtype=np.float32).reshape(N_GLYPHS, PIX)
    assert data.shape == (B, GRID_H, GRID_W, 264), data.shape

    f8 = ml_dtypes.float8_e4m3
    # glyph table [256, 128] -> [chmod, half, pix] fp8
    g8 = np.ascontiguousarray(
        glyph.reshape(2, 128, PIX).transpose(1, 0, 2).astype(f8)
    )

    in_maps = []
    for i in range(N_CORES):
        shard = data[i * B_SHARD : (i + 1) * B_SHARD].reshape(CELLS, 264)
        # glyph channels: [t, p, j, half, chmod] -> [chmod, half, t, j, p] fp8
        gl = shard[:, :256].astype(f8).reshape(NT, 128, OCT, 2, 128)
        d8 = np.ascontiguousarray(gl.transpose(4, 3, 0, 2, 1))
        # color channels: [t, p, j, 8] fp16
        cc = np.ascontiguousarray(
            shard[:, 256:264].astype(np.float16).reshape(NT, 128, OCT, 8)
        )
        in_maps.append({"data8": d8, "datac": cc, "glyph8": g8})

    nc = _get_nc()
    if trace:
        _ensure_ntff_hook()
    res = run_bass_kernel_spmd(
        nc, in_maps, core_ids=list(range(N_CORES)), trace=trace
    )
    outs = []
    for r in res.results:
        # [t, p, j, c, y, x] -> [t, rg, o, j, c, y, x] -> [t, rg, y, o, j, x, c]
        o = r["out"].reshape(NT, 32, 4, OCT, 3, GLYPH_H, GLYPH_W)
        o = o.transpose(0, 1, 5, 2, 3, 6, 4).astype(np.float32)
        # rows r = t*32+rg = (b, h); W = o*160 + j*8 + x
        o = o.reshape(B_SHARD, GRID_H * GLYPH_H, GRID_W * GLYPH_W, 3)
        outs.append(o)
    out = np.concatenate(outs, axis=0)
    return out, res.exec_time_ns


def kernel(data, char_matrix):
    out, _ = run(data, char_matrix, trace=False)
    return out


# revision 13
# speedup vs baseline: 1.2439x; 1.0462x over previous
"""Trainium2 Bass kernel for AnsiToPixels (embedding_lookup, memory-bound).

Computation (per glyph cell):
  raw[y,x]  = sum_ch char[ch] * glyph[ch,y,x]          (256-ch dense "one-hot" matmul)
  fg[c]     = (0.5*fg_bold+0.5) * fg_color[c]
  bg[c]     = (0.5*bg_bold+0.5) * bg_color[c]
  out[y,x,c] = raw[y,x]*(fg[c]-bg[c]) + bg[c]

Sharding: pure data parallelism over batch B=128 -> 16 per core on 8 cores,
glyph table replicated. Each core processes 25600 cells.

Design notes (v4):
  - Host uploads the 256 glyph channels CHANNEL-MAJOR as fp8e4m3
    ([chmod, half, t, j, p]) so the kernel needs NO PE transposes and no
    PSUM->SBUF ct copies; the 8 color channels ride separately as fp16
    cell-major. Input HBM traffic ~7 MB/core; output fp16 ~19.7 MB/core.
  - Glyph lookup: ONE DoubleRow fp8 matmul per 128-cell column (K=256 as
    2 k-tiles), all 20 matmuls of a macro-tile emitted back-to-back so
    the PE pipelines and ramps its clock.
  - Blend out = raw*d + bgs is assigned per 4-cell GROUP to one engine
    to minimize cross-engine semaphore traffic (the NC's activity
    throttle clamps utilization to 50% for most of the kernel, making
    per-instruction overhead the dominant cost):
      * group 0 of each tile: ACT, 12 small per-(j,c) activations
        reading f32 PSUM directly (scale=d, bias=bgs).
      * groups 1-4: DVE big tensor_tensor mult (raw bcast over c x d
        bcast over pix) from PSUM into fp16 out_sb, then an in-place
        broadcast add of bgs on Pool (groups 1,2[,3]) or DVE (rest).
    This is ~2 big instrs/group instead of 12 small ones.
  - Color math (d = fg*sf - bg*sb, bgs) is 3 merged Pool ops per tile,
    computed one tile ahead; d/bgs stored fp16.
  - Output DRAM layout mirrors SBUF ([tile, p, j, c, y, x] fp16), one
    DMA per group; host does the final permutation to [B, 320, 640, 3].
"""

import os
import sys

import numpy as np

for _p in ("/opt/trn_rl_repo", "/root/.axon_site/_ro/trn_rl_repo"):
    if os.path.isdir(_p) and _p not in sys.path:
        sys.path.insert(0, _p)

import ml_dtypes  # noqa: E402

import concourse.bass as bass  # noqa: E402
import concourse.mybir as mybir  # noqa: E402
import concourse.tile as tile  # noqa: E402
from concourse import bacc  # noqa: E402
from concourse.bass_utils import run_bass_kernel_spmd  # noqa: E402


def _ensure_ntff_hook():
    """Register the axon NTFF profile hook if the image's antenv lacks it,
    so run_bass_kernel_spmd(trace=True) can capture HW exec time."""
    try:
        from antenv.axon_hooks import get_axon_ntff_profile_hook  # noqa: F401

        return
    except ImportError:
        pass
    try:
        import types

        import antenv
        from trn_agent_boot.trn_boot import _ntff_profile_via_ctypes

        hook = _ntff_profile_via_ctypes("/opt/axon/libaxon_pjrt.so")
        mod = types.ModuleType("antenv.axon_hooks")
        mod.get_axon_ntff_profile_hook = lambda: hook
        mod.set_axon_ntff_profile_hook = lambda h: None
        sys.modules["antenv.axon_hooks"] = mod
        antenv.axon_hooks = mod
    except Exception as e:  # profiling is best-effort
        print(f"NTFF hook registration failed: {e}", file=sys.stderr)


N_CORES = 8
B = 128
GRID_H, GRID_W = 20, 80
GLYPH_H, GLYPH_W = 16, 8
N_GLYPHS = 256
PIX = GLYPH_H * GLYPH_W  # 128

B_SHARD = B // N_CORES  # 16
CELLS = B_SHARD * GRID_H * GRID_W  # 25600
OCT = 20  # cells per partition per macro-tile
MT = 128 * OCT  # cells per macro-tile (2560)
NT = CELLS // MT  # 10 macro-tiles
NG = OCT // 4  # j-groups per macro-tile (5)

F32 = mybir.dt.float32
F16 = mybir.dt.float16
F8 = mybir.dt.float8e4


def _view(ap, dims):
    """AP with the partition dim of `ap` and explicit free dims [(stride, n)].
    Strides are in elements of the tile dtype; offset comes from `ap`."""
    return bass.AP(
        tensor=ap.tensor, offset=ap.offset, ap=[ap.ap[0], *[[s, n] for s, n in dims]]
    )


def build_kernel():
    nc = bacc.Bacc(
        "TRN2",
        target_bir_lowering=False,
        debug=False,
        enable_asserts=False,
        num_devices=N_CORES,
    )
    # glyph channels, channel-major fp8: [chmod, half, t, j, pcol]
    data8 = nc.dram_tensor(
        "data8", [128, 2, NT, OCT, 128], F8, kind="ExternalInput"
    ).ap()
    # color channels, cell-major fp16: [t, p, j, 8]
    datac = nc.dram_tensor("datac", [NT, 128, OCT, 8], F16, kind="ExternalInput").ap()
    # glyph table fp8: [chmod, half, pix]
    glyph8 = nc.dram_tensor("glyph8", [128, 2, PIX], F8, kind="ExternalInput").ap()
    outp = nc.dram_tensor(
        "out", [NT, 128, OCT, 3, PIX], F16, kind="ExternalOutput"
    ).ap()

    with tile.TileContext(nc) as tc:
        with (
            tc.tile_pool(name="const", bufs=1) as const,
            tc.tile_pool(name="ct", bufs=3) as ct_pool,
            tc.tile_pool(name="cc", bufs=3) as cc_pool,
            tc.tile_pool(name="grp", bufs=3) as grp_pool,
            tc.tile_pool(name="raw", bufs=3) as raw_pool,
            tc.tile_pool(name="outsb", bufs=3) as out_pool,
            tc.tile_pool(name="psR", bufs=6, space="PSUM") as psR,
        ):
            g8 = const.tile([128, 2, PIX], F8)
            nc.sync.dma_start(out=g8[:, :, :], in_=glyph8[:, :, :])
            half_c = const.tile([128, 1], F32)
            nc.gpsimd.memset(half_c[:, :], 0.5)

            cts = {}
            ccs = {}

            def load_tile(t):
                cts[t] = ct_pool.tile([128, 2, OCT, 128], F8, name="ct", tag="ct")
                nc.sync.dma_start(out=cts[t][:, :, :, :], in_=data8[:, :, t, :, :])
                ccs[t] = cc_pool.tile([128, OCT, 8], F16, name="cc", tag="cc")
                nc.sync.dma_start(out=ccs[t][:, :, :], in_=datac[t, :, :, :])

            colors = {}

            def color_math(t):
                """d = fg*sf - bg*sb; bgs = bg*sb. sfsb on ACT, rest Pool."""
                cc = ccs[t]
                sfsb = grp_pool.tile([128, OCT, 2], F32, tag="sfsb")
                fgbg = grp_pool.tile([128, OCT, 2, 3], F32, tag="fgbg")
                d = grp_pool.tile([128, OCT, 3], F32, tag="d")
                # sfsb[:, j, h] = 0.5*cc[:, j, 4h] + 0.5  (ACT affine)
                nc.scalar.activation(
                    sfsb[:, :, :],
                    _view(cc[:, :, :], [(8, OCT), (4, 2)]),
                    mybir.ActivationFunctionType.Identity,
                    bias=half_c[:, :],
                    scale=0.5,
                )
                # fgbg[:, j, h, c] = cc[:, j, 1+4h+c] * sfsb[:, j, h]
                cc1 = cc[:, :, 1:4]  # offset anchor
                nc.gpsimd.tensor_mul(
                    fgbg[:, :, :, :],
                    _view(cc1, [(8, OCT), (4, 2), (1, 3)]),
                    _view(sfsb[:, :, :], [(2, OCT), (1, 2), (0, 3)]),
                )
                # d = fg_s - bg_s
                nc.gpsimd.tensor_sub(d[:, :, :], fgbg[:, :, 0, :], fgbg[:, :, 1, :])
                colors[t] = (d, fgbg)

            load_tile(0)
            color_math(0)
            load_tile(1)

            # greedy static load balancer (estimated us per engine)
            load = {"v": 0.0, "p": 0.0, "a": 0.0}
            # (p1_owner, p2_owner): deltas per engine
            CHOICES = [
                (("a", None), {"a": 5.5}),
                (("v", "v"), {"v": 3.6}),
                (("v", "p"), {"v": 1.8, "p": 1.7}),
                (("q", "v"), {"p": 3.0, "a": 0.63, "v": 1.8}),
                (("q", "p"), {"p": 4.7, "a": 0.63}),
            ]

            def pick():
                best, bestm = None, None
                for ch, dl in CHOICES:
                    m = max(load[e] + dl.get(e, 0.0) for e in load)
                    if bestm is None or m < bestm:
                        best, bestm = (ch, dl), m
                ch, dl = best
                for e, v in dl.items():
                    load[e] += v
                return ch

            for t in range(NT):
                if t + 1 < NT:
                    color_math(t + 1)
                if t + 2 < NT:
                    load_tile(t + 2)
                ct = cts.pop(t)
                ccs.pop(t)
                d, fgbg = colors.pop(t)
                # bgs lives inside fgbg at [:, j, 1, c]

                out_sb = out_pool.tile([128, OCT, 3, PIX], F16)

                # all 20 matmuls back-to-back so the PE pipelines/ramps
                rawps = []
                for g in range(NG):
                    rawp = psR.tile([128, 4 * PIX], F32)
                    rawps.append(rawp)
                    j0 = 4 * g
                    for jj in range(4):
                        nc.tensor.matmul(
                            rawp[:, jj * PIX : (jj + 1) * PIX],
                            ct[:, :, j0 + jj, :],
                            g8[:, :, :],
                            start=True,
                            stop=True,
                            perf_mode=mybir.MatmulPerfMode.DoubleRow,
                        )

                # per-tile fixed engine work (color math emitted above)
                load["p"] += 2.1
                load["a"] += 0.35

                for g in range(NG):
                    j0 = 4 * g
                    rawp = rawps[g]
                    out_g = out_sb[:, j0 : j0 + 4, :, :]
                    p1, p2 = pick()
                    if p1 == "a":
                        # ACT group: 12 small activations from PSUM
                        for jj in range(4):
                            j = j0 + jj
                            for c in range(3):
                                nc.scalar.activation(
                                    out_sb[:, j, c, :],
                                    rawp[:, jj * PIX : (jj + 1) * PIX],
                                    mybir.ActivationFunctionType.Identity,
                                    bias=fgbg[:, j, 1, c : c + 1],
                                    scale=d[:, j, c : c + 1],
                                )
                    else:
                        d_bc = _view(
                            d[:, j0 : j0 + 4, :], [(3, 4), (1, 3), (0, PIX)]
                        )
                        if p1 == "v":
                            # pass1 (DVE): out = raw (bcast c) * d (bcast pix)
                            nc.vector.tensor_mul(
                                out_g,
                                _view(rawp[:, :], [(PIX, 4), (0, 3), (1, PIX)]),
                                d_bc,
                            )
                        else:
                            # Pool can't read PSUM: ACT copies raw to fp16 SBUF
                            raw16 = raw_pool.tile([128, 4 * PIX], F16)
                            nc.scalar.copy(raw16[:, :], rawp[:, :])
                            nc.gpsimd.tensor_mul(
                                out_g,
                                _view(raw16[:, :], [(PIX, 4), (0, 3), (1, PIX)]),
                                d_bc,
                            )
                        # pass2: out += bgs (bcast pix)
                        e = nc.vector if p2 == "v" else nc.gpsimd
                        e.tensor_add(
                            out_g,
                            out_g,
                            _view(
                                fgbg[:, j0 : j0 + 4, 1, :], [(6, 4), (1, 3), (0, PIX)]
                            ),
                        )
                    nc.sync.dma_start(
                        out=outp[t, :, j0 : j0 + 4, :, :],
                        in_=out_g,
                    )

    nc.compile()
    return nc


_NC = None


def _get_nc():
    global _NC
    if _NC is None:
        _NC = build_kernel()
    return _NC


def run(data, char_matrix, trace=False):
    data = np.ascontiguousarray(np.asarray(data, dtype=np.float32))
    glyph = np.asarray(char_matrix, dtype=np.float32).reshape(N_GLYPHS, PIX)
    assert data.shape == (B, GRID_H, GRID_W, 264), data.shape

    f8 = ml_dtypes.float8_e4m3
    # glyph table [256, 128] -> [chmod, half, pix] fp8
    g8 = np.ascontiguousarray(
        glyph.reshape(2, 128, PIX).transpose(1, 0, 2).astype(f8)
    )

    in_maps = []
    for i in range(N_CORES):
        shard = data[i * B_SHARD : (i + 1) * B_SHARD].reshape(CELLS, 264)
        # glyph channels: [t, p, j, half, chmod] -> [chmod, half, t, j, p] fp8
        gl = shard[:, :256].astype(f8).reshape(NT, 128, OCT, 2, 128)
        d8 = np.ascontiguousarray(gl.transpose(4, 3, 0, 2, 1))
        # color channels: [t, p, j, 8] fp16
        cc = np.ascontiguousarray(
            shard[:, 256:264].astype(np.float16).reshape(NT, 128, OCT, 8)
        )
        in_maps.append({"data8": d8, "datac": cc, "glyph8": g8})

    nc = _get_nc()
    if trace:
        _ensure_ntff_hook()
    res = run_bass_kernel_spmd(
        nc, in_maps, core_ids=list(range(N_CORES)), trace=trace
    )
    outs = []
    for r in res.results:
        # [t, p, j, c, y, x] -> [t, rg, o, j, c, y, x] -> [t, rg, y, o, j, x, c]
        o = r["out"].reshape(NT, 32, 4, OCT, 3, GLYPH_H, GLYPH_W)
        o = o.transpose(0, 1, 5, 2, 3, 6, 4).astype(np.float32)
        # rows r = t*32+rg = (b, h); W = o*160 + j*8 + x
        o = o.reshape(B_SHARD, GRID_H * GLYPH_H, GRID_W * GLYPH_W, 3)
        outs.append(o)
    out = np.concatenate(outs, axis=0)
    return out, res.exec_time_ns


def kernel(data, char_matrix):
    out, _ = run(data, char_matrix, trace=False)
    return out


# revision 17
# speedup vs baseline: 1.3723x; 1.1033x over previous
"""Trainium2 Bass kernel for AnsiToPixels (embedding_lookup, memory-bound).

Computation (per glyph cell):
  raw[y,x]  = sum_ch char[ch] * glyph[ch,y,x]          (256-ch dense "one-hot" matmul)
  fg[c]     = (0.5*fg_bold+0.5) * fg_color[c]
  bg[c]     = (0.5*bg_bold+0.5) * bg_color[c]
  out[y,x,c] = raw[y,x]*(fg[c]-bg[c]) + bg[c]

Sharding: pure data parallelism over batch B=128 -> 16 per core on 8 cores,
glyph table replicated. Each core processes 25600 cells.

Design notes (v4):
  - Host uploads the 256 glyph channels CHANNEL-MAJOR as fp8e4m3
    ([chmod, half, t, j, p]) so the kernel needs NO PE transposes and no
    PSUM->SBUF ct copies; the 8 color channels ride separately as fp16
    cell-major. Input HBM traffic ~7 MB/core; output fp16 ~19.7 MB/core.
  - Glyph lookup: ONE DoubleRow fp8 matmul per 128-cell column (K=256 as
    2 k-tiles), all 20 matmuls of a macro-tile emitted back-to-back so
    the PE pipelines and ramps its clock.
  - Blend out = raw*d + bgs is assigned per 4-cell GROUP to one engine
    to minimize cross-engine semaphore traffic (the NC's activity
    throttle clamps utilization to 50% for most of the kernel, making
    per-instruction overhead the dominant cost):
      * group 0 of each tile: ACT, 12 small per-(j,c) activations
        reading f32 PSUM directly (scale=d, bias=bgs).
      * groups 1-4: DVE big tensor_tensor mult (raw bcast over c x d
        bcast over pix) from PSUM into fp16 out_sb, then an in-place
        broadcast add of bgs on Pool (groups 1,2[,3]) or DVE (rest).
    This is ~2 big instrs/group instead of 12 small ones.
  - Color math (d = fg*sf - bg*sb, bgs) is 3 merged Pool ops per tile,
    computed one tile ahead; d/bgs stored fp16.
  - Output DRAM layout mirrors SBUF ([tile, p, j, c, y, x] fp16), one
    DMA per group; host does the final permutation to [B, 320, 640, 3].
"""

import os
import sys

import numpy as np

for _p in ("/opt/trn_rl_repo", "/root/.axon_site/_ro/trn_rl_repo"):
    if os.path.isdir(_p) and _p not in sys.path:
        sys.path.insert(0, _p)

import ml_dtypes  # noqa: E402

import concourse.bass as bass  # noqa: E402
import concourse.mybir as mybir  # noqa: E402
import concourse.tile as tile  # noqa: E402
from concourse import bacc  # noqa: E402
from concourse.bass_utils import run_bass_kernel_spmd  # noqa: E402


def _ensure_ntff_hook():
    """Register the axon NTFF profile hook if the image's antenv lacks it,
    so run_bass_kernel_spmd(trace=True) can capture HW exec time."""
    try:
        from antenv.axon_hooks import get_axon_ntff_profile_hook  # noqa: F401

        return
    except ImportError:
        pass
    try:
        import types

        import antenv
        from trn_agent_boot.trn_boot import _ntff_profile_via_ctypes

        hook = _ntff_profile_via_ctypes("/opt/axon/libaxon_pjrt.so")
        mod = types.ModuleType("antenv.axon_hooks")
        mod.get_axon_ntff_profile_hook = lambda: hook
        mod.set_axon_ntff_profile_hook = lambda h: None
        sys.modules["antenv.axon_hooks"] = mod
        antenv.axon_hooks = mod
    except Exception as e:  # profiling is best-effort
        print(f"NTFF hook registration failed: {e}", file=sys.stderr)


N_CORES = 8
B = 128
GRID_H, GRID_W = 20, 80
GLYPH_H, GLYPH_W = 16, 8
N_GLYPHS = 256
PIX = GLYPH_H * GLYPH_W  # 128

B_SHARD = B // N_CORES  # 16
CELLS = B_SHARD * GRID_H * GRID_W  # 25600
OCT = 20  # cells per partition per macro-tile
MT = 128 * OCT  # cells per macro-tile (2560)
NT = CELLS // MT  # 10 macro-tiles
NG = OCT // 4  # j-groups per macro-tile (5)

F32 = mybir.dt.float32
F16 = mybir.dt.float16
F8 = mybir.dt.float8e4


def _view(ap, dims):
    """AP with the partition dim of `ap` and explicit free dims [(stride, n)].
    Strides are in elements of the tile dtype; offset comes from `ap`."""
    return bass.AP(
        tensor=ap.tensor, offset=ap.offset, ap=[ap.ap[0], *[[s, n] for s, n in dims]]
    )


def build_kernel():
    nc = bacc.Bacc(
        "TRN2",
        target_bir_lowering=False,
        debug=False,
        enable_asserts=False,
        num_devices=N_CORES,
    )
    # glyph channels, channel-major fp8: [chmod, half, t, j, pcol]
    data8 = nc.dram_tensor(
        "data8", [128, 2, NT, OCT, 128], F8, kind="ExternalInput"
    ).ap()
    # color channels, cell-major fp16: [t, p, j, 8]
    datac = nc.dram_tensor("datac", [NT, 128, OCT, 8], F16, kind="ExternalInput").ap()
    # glyph table fp8: [chmod, half, pix]
    glyph8 = nc.dram_tensor("glyph8", [128, 2, PIX], F8, kind="ExternalInput").ap()
    outp = nc.dram_tensor(
        "out", [NT, 128, OCT, 3, PIX], F16, kind="ExternalOutput"
    ).ap()

    with tile.TileContext(nc) as tc:
        with (
            tc.tile_pool(name="const", bufs=1) as const,
            tc.tile_pool(name="ct", bufs=3) as ct_pool,
            tc.tile_pool(name="cc", bufs=3) as cc_pool,
            tc.tile_pool(name="grp", bufs=3) as grp_pool,
            tc.tile_pool(name="outsb", bufs=3) as out_pool,
            tc.tile_pool(name="psR", bufs=8, space="PSUM") as psR,
        ):
            g8 = const.tile([128, 2, PIX], F8)
            nc.sync.dma_start(out=g8[:, :, :], in_=glyph8[:, :, :])
            half_c = const.tile([128, 1], F32)
            nc.gpsimd.memset(half_c[:, :], 0.5)

            cts = {}
            ccs = {}

            def load_tile(t):
                cts[t] = ct_pool.tile([128, 2, OCT, 128], F8, name="ct", tag="ct")
                nc.sync.dma_start(out=cts[t][:, :, :, :], in_=data8[:, :, t, :, :])
                ccs[t] = cc_pool.tile([128, OCT, 8], F16, name="cc", tag="cc")
                nc.sync.dma_start(out=ccs[t][:, :, :], in_=datac[t, :, :, :])

            colors = {}

            def color_math(t):
                """sf/sb on ACT; fgs/bgs/d as 3 Pool STTs (all <=3D)."""
                cc = ccs[t]
                sfsb = grp_pool.tile([128, OCT, 2], F32, tag="sfsb")
                fgs = grp_pool.tile([128, OCT, 3], F32, tag="fgs")
                bgs = grp_pool.tile([128, OCT, 3], F32, tag="bgs")
                d = grp_pool.tile([128, OCT, 3], F32, tag="d")
                # sfsb[:, j, h] = 0.5*cc[:, j, 4h] + 0.5  (ACT affine)
                nc.scalar.activation(
                    sfsb[:, :, :],
                    _view(cc[:, :, :], [(8, OCT), (4, 2)]),
                    mybir.ActivationFunctionType.Identity,
                    bias=half_c[:, :],
                    scale=0.5,
                )
                # fgs = fg_color * sf ; bgs = bg_color * sb
                for h, dst in ((0, fgs), (1, bgs)):
                    nc.gpsimd.tensor_mul(
                        dst[:, :, :],
                        _view(cc[:, :, 1 + 4 * h : 4 + 4 * h], [(8, OCT), (1, 3)]),
                        _view(sfsb[:, :, h : h + 1], [(2, OCT), (0, 3)]),
                    )
                # d = fgs - bgs
                nc.gpsimd.tensor_sub(d[:, :, :], fgs[:, :, :], bgs[:, :, :])
                colors[t] = (d, bgs)

            load_tile(0)
            color_math(0)
            load_tile(1)

            # greedy static load balancer (estimated us per engine)
            load = {"v": 0.0, "p": 0.0, "a": 0.0}
            # (p1_owner, p2_owner): deltas per engine
            CHOICES = [
                (("a", None), {"a": 5.5}),
                (("v", "v"), {"v": 3.68}),
                (("v", "p"), {"v": 1.84, "p": 3.04}),
            ]

            def pick():
                best, bestm = None, None
                for ch, dl in CHOICES:
                    m = max(load[e] + dl.get(e, 0.0) for e in load)
                    if bestm is None or m < bestm:
                        best, bestm = (ch, dl), m
                ch, dl = best
                for e, v in dl.items():
                    load[e] += v
                return ch

            for t in range(NT):
                if t + 1 < NT:
                    color_math(t + 1)
                if t + 2 < NT:
                    load_tile(t + 2)
                ct = cts.pop(t)
                ccs.pop(t)
                d, bgs = colors.pop(t)

                out_sb = out_pool.tile([128, OCT, 3, PIX], F16)

                # all 20 matmuls back-to-back so the PE pipelines/ramps
                rawps = []
                for g in range(NG):
                    rawp = psR.tile([128, 4 * PIX], F32)
                    rawps.append(rawp)
                    j0 = 4 * g
                    for jj in range(4):
                        nc.tensor.matmul(
                            rawp[:, jj * PIX : (jj + 1) * PIX],
                            ct[:, :, j0 + jj, :],
                            g8[:, :, :],
                            start=True,
                            stop=True,
                            perf_mode=mybir.MatmulPerfMode.DoubleRow,
                        )

                # per-tile fixed engine work (color math emitted above)
                load["p"] += 1.0
                load["a"] += 0.35

                for g in range(NG):
                    j0 = 4 * g
                    rawp = rawps[g]
                    out_g = out_sb[:, j0 : j0 + 4, :, :]
                    p1, p2 = pick()
                    if p1 == "a":
                        # ACT group: 12 small activations from PSUM
                        for jj in range(4):
                            j = j0 + jj
                            for c in range(3):
                                nc.scalar.activation(
                                    out_sb[:, j, c, :],
                                    rawp[:, jj * PIX : (jj + 1) * PIX],
                                    mybir.ActivationFunctionType.Identity,
                                    bias=bgs[:, j, c : c + 1],
                                    scale=d[:, j, c : c + 1],
                                )
                    else:
                        # pass1 (DVE): out = raw (bcast c) * d (bcast pix)
                        nc.vector.tensor_mul(
                            out_g,
                            _view(rawp[:, :], [(PIX, 4), (0, 3), (1, PIX)]),
                            _view(d[:, j0 : j0 + 4, :], [(3, 4), (1, 3), (0, PIX)]),
                        )
                        # pass2: out += bgs (bcast pix)
                        if p2 == "v":
                            nc.vector.tensor_add(
                                out_g,
                                out_g,
                                _view(
                                    bgs[:, j0 : j0 + 4, :], [(3, 4), (1, 3), (0, PIX)]
                                ),
                            )
                        else:
                            nc.gpsimd.tensor_add(
                                out_g,
                                out_g,
                                _view(
                                    bgs[:, j0 : j0 + 4, :], [(3, 4), (1, 3), (0, PIX)]
                                ),
                            )
                    nc.sync.dma_start(
                        out=outp[t, :, j0 : j0 + 4, :, :],
                        in_=out_g,
                    )

    nc.compile()
    return nc


_NC = None


def _get_nc():
    global _NC
    if _NC is None:
        _NC = build_kernel()
    return _NC


def run(data, char_matrix, trace=False):
    data = np.ascontiguousarray(np.asarray(data, dtype=np.float32))
    glyph = np.asarray(char_matrix, dtype=np.float32).reshape(N_GLYPHS, PIX)
    assert data.shape == (B, GRID_H, GRID_W, 264), data.shape

    f8 = ml_dtypes.float8_e4m3
    # glyph table [256, 128] -> [chmod, half, pix] fp8
    g8 = np.ascontiguousarray(
        glyph.reshape(2, 128, PIX).transpose(1, 0, 2).astype(f8)
    )

    in_maps = []
    for i in range(N_CORES):
        shard = data[i * B_SHARD : (i + 1) * B_SHARD].reshape(CELLS, 264)
        # glyph channels: [t, p, j, half, chmod] -> [chmod, half, t, j, p] fp8
        gl = shard[:, :256].astype(f8).reshape(NT, 128, OCT, 2, 128)
        d8 = np.ascontiguousarray(gl.transpose(4, 3, 0, 2, 1))
        # color channels: [t, p, j, 8] fp16
        cc = np.ascontiguousarray(
            shard[:, 256:264].astype(np.float16).reshape(NT, 128, OCT, 8)
        )
        in_maps.append({"data8": d8, "datac": cc, "glyph8": g8})

    nc = _get_nc()
    if trace:
        _ensure_ntff_hook()
    res = run_bass_kernel_spmd(
        nc, in_maps, core_ids=list(range(N_CORES)), trace=trace
    )
    outs = []
    for r in res.results:
        # [t, p, j, c, y, x] -> [t, rg, o, j, c, y, x] -> [t, rg, y, o, j, x, c]
        o = r["out"].reshape(NT, 32, 4, OCT, 3, GLYPH_H, GLYPH_W)
        o = o.transpose(0, 1, 5, 2, 3, 6, 4).astype(np.float32)
        # rows r = t*32+rg = (b, h); W = o*160 + j*8 + x
        o = o.reshape(B_SHARD, GRID_H * GLYPH_H, GRID_W * GLYPH_W, 3)
        outs.append(o)
    out = np.concatenate(outs, axis=0)
    return out, res.exec_time_ns


def kernel(data, char_matrix):
    out, _ = run(data, char_matrix, trace=False)
    return out
